# revision 6
# baseline (speedup 1.0000x reference)
"""Trainium2 Bass kernel for nn_Cond_PlanarTrans (conditional planar flow, MoE-routing).

Math (per batch b, particle i):
    w = relu(o @ W1.T + b1).reshape(B, 8, 64)
    u = relu(o @ W2.T + b2).reshape(B, 8, 64)
    bf = relu(o @ W3.T + b3).reshape(B, 8)
    n = m[b, i]
    pre = <s_t[b,i,:], w[b,n,:]> + bf[b,n]
    out[b,i,:] = s_t[b,i,:] + u[b,n,:] * tanh(pre)

Strategy: data-parallel over B across 8 cores (16 batches each). On each core:
  - tiny MLP computed once on the PE (weights transposed on-chip)
  - per-particle gathers are block-diagonal one-hot matmuls: 4 chunks (of 128
    particles) share ONE matmul whose stationary operand is the 4 chunks'
    one-hot masks stacked [32, 128] and whose moving operand is a
    block-diagonal [32, 4*65] fp16 [w|bf] table -> PSUM [128, 4, 65]
  - dot product runs on DVE in packed fp16 (2x): ACT evacuates the gathered
    w_m to fp16 SBUF, s_t is cast-loaded fp16 via SWDGE, one fused 256-col
    multiply + one 260-col reduce per 4-chunk group
  - tanh once per batch [128, 16] on ACT; a PE transpose+replicate turns it
    into rows so DVE can scale the one-hot masks by tanh (exact: oh is 0/1);
    a second block-diagonal matmul then yields u_m * tanh directly in PSUM
  - final add s_t + upd on GPSIMD (fp16 inputs, fp32 out), stores on the
    sync HWDGE queue while s_t cast-loads ride the SWDGE queue.

Particle layout: partition p of a batch holds particles 16p..16p+15 (contiguous
4KB per partition -> full-rate DMA); chunk j of a batch = particles {16p+j}.
One-hot host layout: oh_jn[b, j*8+n, p] = (m[b, 16p+j] == n), fp16 [B, 128, 128].
"""

import os
import sys

import numpy as np

B, P, DIM, N_M = 128, 2048, 64, 8
NCORES = 8
BL = B // NCORES  # batches per core
JC = 16           # chunks per batch (particle = 16*p + j)
G = 4             # chunks per group (one gather matmul per group)
NG = JC // G      # groups per batch
WC = DIM + 1      # 65: [w | bf]

# tunables
NT = int(os.environ.get("PK_NT", "3"))    # s_t tile ring depth
OB = int(os.environ.get("PK_OB", "3"))    # out tile bufs
ADD_ENG = os.environ.get("PK_ADD", "gpsimd")  # gpsimd | vector

LAST_EXEC_NS = None
LAST_RESULTS = None

_CACHE = {}


def _import_concourse():
    try:
        import concourse.bass  # noqa: F401
    except ImportError:
        for p in ("/opt/trn_rl_repo", "/root/.axon_site/_ro/trn_rl_repo"):
            if os.path.isdir(p) and p not in sys.path:
                sys.path.insert(0, p)
        import concourse.bass  # noqa: F401


def _ensure_ntff_hook():
    """Provide antenv.axon_hooks (get/set_axon_ntff_profile_hook) if the image
    lacks it, wiring the NTFF profile capture directly to libaxon_pjrt.so."""
    try:
        from antenv.axon_hooks import get_axon_ntff_profile_hook  # noqa: F401
        return
    except ImportError:
        pass

    import contextlib
    import ctypes
    import types

    so_path = os.environ.get("AXON_PJRT_SO", "/opt/axon/libaxon_pjrt.so")
    hook = None
    if os.path.exists(so_path):
        lib = ctypes.CDLL(so_path)
        if hasattr(lib, "axon_start_nrt_profile"):
            lib.axon_start_nrt_profile.argtypes = [
                ctypes.POINTER(ctypes.c_int64),
                ctypes.c_size_t,
            ]
            lib.axon_start_nrt_profile.restype = ctypes.c_int64
            lib.axon_stop_nrt_profile.argtypes = [ctypes.c_char_p]
            lib.axon_stop_nrt_profile.restype = ctypes.c_int64

            @contextlib.contextmanager
            def hook(output_dir, device_ids):  # noqa: F811
                import jax

                jax.devices()
                if device_ids:
                    ids = (ctypes.c_int64 * len(device_ids))(*device_ids)
                    rc = lib.axon_start_nrt_profile(ids, len(device_ids))
                else:
                    rc = lib.axon_start_nrt_profile(None, 0)
                if rc != 0:
                    raise RuntimeError(f"axon_start_nrt_profile rc={rc}")
                try:
                    yield
                finally:
                    n = lib.axon_stop_nrt_profile(str(output_dir).encode())
                    print(f"profile: {n} file(s) written to {output_dir}")

    state = {"hook": hook}
    mod = types.ModuleType("antenv.axon_hooks")
    mod.get_axon_ntff_profile_hook = lambda: state["hook"]

    def _set(h):
        state["hook"] = h

    mod.set_axon_ntff_profile_hook = _set
    import antenv

    antenv.axon_hooks = mod
    sys.modules["antenv.axon_hooks"] = mod


def _build_bass():
    _import_concourse()
    from contextlib import ExitStack

    import concourse.bacc as bacc
    import concourse.bass as bass  # noqa: F401
    import concourse.tile as tile
    from concourse import mybir
    from concourse.masks import make_identity

    f32 = mybir.dt.float32
    f16 = mybir.dt.float16
    AF = mybir.ActivationFunctionType
    OP = mybir.AluOpType
    AX = mybir.AxisListType

    # Bacc (not plain Bass): its finalize() splits multi-sem waits into event
    # semaphores — TRN2 instructions carry at most one wait.
    nc = bacc.Bacc(None)

    s_t = nc.declare_dram_parameter("s_t", [BL, P, DIM], f32, isOutput=False)
    ohjn = nc.declare_dram_parameter("ohjn", [BL, 96, 2, 128], f16, isOutput=False)
    o_in = nc.declare_dram_parameter("o", [BL, DIM], f32, isOutput=False)
    W1 = nc.declare_dram_parameter("W1", [N_M * DIM, DIM], f32, isOutput=False)
    b1 = nc.declare_dram_parameter("b1", [N_M * DIM], f32, isOutput=False)
    W2 = nc.declare_dram_parameter("W2", [N_M * DIM, DIM], f32, isOutput=False)
    b2 = nc.declare_dram_parameter("b2", [N_M * DIM], f32, isOutput=False)
    W3 = nc.declare_dram_parameter("W3", [N_M, DIM], f32, isOutput=False)
    b3 = nc.declare_dram_parameter("b3", [N_M], f32, isOutput=False)
    out = nc.declare_dram_parameter("out", [BL, P, DIM], f32, isOutput=True)

    with tile.TileContext(nc) as tc, ExitStack() as ctx:
        consts = ctx.enter_context(tc.tile_pool(name="consts", bufs=1))

        # ---------- phase 0: constants + per-batch MLP ----------
        ident = consts.tile([128, 128], f32)
        make_identity(nc, ident)
        ident16 = consts.tile([128, 128], f16)
        make_identity(nc, ident16)
        ones_row = consts.tile([1, 128], f32)
        nc.vector.memset(ones_row, 1.0)

        w1_sb = consts.tile([128, 4, DIM], f32)
        nc.sync.dma_start(out=w1_sb, in_=W1[:].rearrange("(q r) k -> r q k", r=128))
        w2_sb = consts.tile([128, 4, DIM], f32)
        nc.sync.dma_start(out=w2_sb, in_=W2[:].rearrange("(q r) k -> r q k", r=128))
        w3_sb = consts.tile([N_M, DIM], f32)
        nc.sync.dma_start(out=w3_sb, in_=W3[:])
        b1_sb = consts.tile([1, N_M * DIM], f32)
        nc.sync.dma_start(out=b1_sb, in_=b1[:].rearrange("(a n) -> a n", a=1))
        b2_sb = consts.tile([1, N_M * DIM], f32)
        nc.sync.dma_start(out=b2_sb, in_=b2[:].rearrange("(a n) -> a n", a=1))
        b3_sb = consts.tile([1, N_M], f32)
        nc.sync.dma_start(out=b3_sb, in_=b3[:].rearrange("(a n) -> a n", a=1))
        o_sb = consts.tile([BL, DIM], f32)
        nc.sync.dma_start(out=o_sb, in_=o_in[:])

        with tc.tile_pool(name="mlp_ps", bufs=2, space="PSUM") as mlp_ps:
            # transposes: oT [64, BL]; W1T/W2T [64, 512]; W3T [64, 8]
            oT = consts.tile([DIM, BL], f32)
            pt_o = mlp_ps.tile([DIM, BL], f32, tag="pt")
            nc.tensor.transpose(pt_o, o_sb, ident[0:BL, 0:BL])
            nc.vector.tensor_copy(oT, pt_o)

            w1T = consts.tile([DIM, N_M * DIM], f32)
            w2T = consts.tile([DIM, N_M * DIM], f32)
            for src, dst in ((w1_sb, w1T), (w2_sb, w2T)):
                for q in range(4):
                    pt = mlp_ps.tile([DIM, 128], f32, tag="pt")
                    nc.tensor.transpose(pt, src[:, q, :], ident)
                    nc.vector.tensor_copy(dst[:, q * 128:(q + 1) * 128], pt)
            w3T = consts.tile([DIM, N_M], f32)
            pt_3 = mlp_ps.tile([DIM, N_M], f32, tag="pt")
            nc.tensor.transpose(pt_3, w3_sb, ident[0:N_M, 0:N_M])
            nc.vector.tensor_copy(w3T, pt_3)

            # MLP: x_all = relu(o @ W.T + b), bias preloaded via ones-matmul
            w_all = consts.tile([BL, N_M * DIM], f32)
            u_all = consts.tile([BL, N_M * DIM], f32)
            bf_all = consts.tile([BL, N_M], f32)
            for bsb, wT, dst in (
                (b1_sb, w1T, w_all),
                (b2_sb, w2T, u_all),
                (b3_sb, w3T, bf_all),
            ):
                n_cols = dst.shape[-1]
                ps = mlp_ps.tile([BL, n_cols], f32, tag="mlp")
                nc.tensor.matmul(ps, lhsT=ones_row[0:1, 0:BL], rhs=bsb,
                                 start=True, stop=False)
                nc.tensor.matmul(ps, lhsT=oT, rhs=wT, start=False, stop=True)
                nc.scalar.activation(out=dst, in_=ps, func=AF.Relu)

        # Block-diagonal fp16 gather tables, built via a DRAM bounce
        # (partition-reshape) with inline fp32->fp16 SWDGE cast:
        #   wbf4_sb[8jj+n, b, 65jj+c] = w[b, n, c]   (c<64), bf[b, n] at c=64
        #   u4_sb [8jj+n, b, 64jj+c] = u[b, n, c]
        w_dram = nc.dram_tensor("w_scratch", [BL, N_M * DIM], f32)
        u_dram = nc.dram_tensor("u_scratch", [BL, N_M * DIM], f32)
        bf_dram = nc.dram_tensor("bf_scratch", [BL, N_M], f32)
        nc.sync.dma_start(out=w_dram[:], in_=w_all)
        nc.sync.dma_start(out=u_dram[:], in_=u_all)
        nc.sync.dma_start(out=bf_dram[:], in_=bf_all)

        # [96, ...]: the block-diag table replicated across the three 32-row
        # partition strips (matmul operands may only base at 0/32/64; group 3
        # reuses strip 0), so each group's matmul finds its moving operand at
        # the same base partition as its stationary one-hot slice.
        wbf4_sb = consts.tile([96, BL, G * WC], f16)
        u4_sb = consts.tile([96, BL, G * DIM], f16)
        nc.gpsimd.memset(wbf4_sb[0:32], 0.0)
        nc.gpsimd.memset(u4_sb[0:32], 0.0)
        for jj in range(G):
            nc.gpsimd.dma_start(
                out=wbf4_sb[8 * jj:8 * jj + 8, :, WC * jj:WC * jj + DIM],
                in_=w_dram[:].rearrange("b (n k) -> n b k", n=N_M),
            )
            nc.gpsimd.dma_start(
                out=wbf4_sb[8 * jj:8 * jj + 8, :, WC * jj + DIM:WC * jj + WC],
                in_=bf_dram[:].rearrange("b (n a) -> n b a", a=1),
            )
            nc.gpsimd.dma_start(
                out=u4_sb[8 * jj:8 * jj + 8, :, DIM * jj:DIM * jj + DIM],
                in_=u_dram[:].rearrange("b (n k) -> n b k", n=N_M),
            )
        for gg in range(1, 3):
            nc.sync.dma_start(out=wbf4_sb[32 * gg:32 * gg + 32], in_=wbf4_sb[0:32])
            nc.sync.dma_start(out=u4_sb[32 * gg:32 * gg + 32], in_=u4_sb[0:32])

        def oh_slice(t, g):
            # groups 0-2 live at partition strips 0/32/64 of free-slot 0;
            # group 3 at partitions 0-31 of free-slot 1
            if g < 3:
                return t[32 * g:32 * g + 32, 0, :]
            return t[0:32, 1, :]

        def tbl_slice(t, g, b):
            return t[32 * g:32 * g + 32, b, :] if g < 3 else t[0:32, b, :]

        # ---------- phase 1: main loop ----------
        stpool = ctx.enter_context(tc.tile_pool(name="stpool", bufs=NT))
        ohpool = ctx.enter_context(tc.tile_pool(name="ohpool", bufs=3))
        outpool = ctx.enter_context(tc.tile_pool(name="outpool", bufs=OB))
        wmpool = ctx.enter_context(tc.tile_pool(name="wmpool", bufs=3))
        prpool = ctx.enter_context(tc.tile_pool(name="prpool", bufs=3))
        updpool = ctx.enter_context(tc.tile_pool(name="updpool", bufs=4))
        smpool = ctx.enter_context(tc.tile_pool(name="smpool", bufs=3))
        pswpool = ctx.enter_context(tc.tile_pool(name="psw", bufs=2, space="PSUM"))
        psupool = ctx.enter_context(tc.tile_pool(name="psu", bufs=3, space="PSUM"))
        psthpool = ctx.enter_context(tc.tile_pool(name="psth", bufs=2, space="PSUM"))

        add_eng = nc.gpsimd if ADD_ENG == "gpsimd" else nc.vector

        def emit_phase2_group(prev, g):
            pb, pst, pohs, pout = prev
            psu = psupool.tile([128, G, DIM], f32, tag="psu")
            nc.tensor.matmul(
                psu, lhsT=oh_slice(pohs, g), rhs=tbl_slice(u4_sb, g, pb),
                start=True, stop=True,
            )
            upd = updpool.tile([128, G, DIM], f16, tag="upd")
            nc.scalar.activation(out=upd, in_=psu, func=AF.Copy)
            add_eng.tensor_tensor(
                out=pout[:, G * g:G * g + G, :],
                in0=upd, in1=pst[:, G * g:G * g + G, :], op=OP.add,
            )
            if g == NG - 1:
                nc.sync.dma_start(
                    out=out[pb].rearrange("(p j) k -> p j k", j=JC),
                    in_=pout,
                )

        prev = None  # (b, s_t16, ohs16, out_tile)
        for b in range(BL):
            st = stpool.tile([128, JC, DIM], f16, tag="st")
            nc.gpsimd.dma_start(
                out=st, in_=s_t[b].rearrange("(p j) k -> p j k", j=JC),
            )
            oht = ohpool.tile([96, 2, 128], f16, tag="oh")
            nc.sync.dma_start(out=oht, in_=ohjn[b])
            outt = outpool.tile([128, JC, DIM], f32, tag="outt")
            pre_b = smpool.tile([128, JC], f32, tag="pre")

            for g in range(NG):
                psw = pswpool.tile([128, G, WC], f32, tag="psw")
                nc.tensor.matmul(
                    psw, lhsT=oh_slice(oht, g), rhs=tbl_slice(wbf4_sb, g, b),
                    start=True, stop=True,
                )
                wm = wmpool.tile([128, G, WC + 1], f16, tag="wm")
                nc.scalar.activation(out=wm[:, :, 0:WC], in_=psw, func=AF.Copy)
                pr = prpool.tile([128, G, WC + 1], f16, tag="pr")
                nc.vector.tensor_tensor(
                    out=pr[:, :, 0:DIM], in0=st[:, G * g:G * g + G, :],
                    in1=wm[:, :, 0:DIM], op=OP.mult,
                )
                nc.vector.tensor_copy(pr[:, :, DIM:WC], wm[:, :, DIM:WC])
                nc.vector.reduce_sum(
                    out=pre_b[:, G * g:G * g + G], in_=pr[:, :, 0:WC], axis=AX.X,
                )
                if prev is not None:
                    emit_phase2_group(prev, g)

            th_b = smpool.tile([128, JC], f32, tag="th")
            nc.scalar.activation(out=th_b, in_=pre_b, func=AF.Tanh)
            # replicate along n then transpose: th16[(j n), p] = tanh(pre[p, j])
            thx = smpool.tile([128, JC, N_M], f16, tag="thx")
            th_src = bass.AP(
                tensor=th_b.tensor,
                offset=th_b.offset,
                ap=[th_b.ap[0], [th_b.ap[1][0], JC], [0, N_M]],
            )
            nc.vector.tensor_copy(thx, th_src)
            psth = psthpool.tile([96, 2, 128], f16, tag="psth")
            thxf = thx.rearrange("p j n -> p (j n)")
            nc.tensor.transpose(psth[0:96, 0, :], thxf[:, 0:96], ident16)
            nc.tensor.transpose(psth[0:32, 1, :], thxf[:, 96:128],
                                ident16[0:128, 0:128])
            ohs = smpool.tile([96, 2, 128], f16, tag="ohs")
            nc.vector.tensor_tensor(out=ohs, in0=oht, in1=psth, op=OP.mult)

            prev = (b, st, ohs, outt)

        for g in range(NG):
            emit_phase2_group(prev, g)

    nc.finalize()
    return nc


def _get_bass():
    if "nc" not in _CACHE:
        _CACHE["nc"] = _build_bass()
    return _CACHE["nc"]


def kernel(m, s_t, o, W1, b1, W2, b2, W3, b3):
    global LAST_EXEC_NS, LAST_RESULTS
    _import_concourse()
    from concourse.bass_utils import run_bass_kernel_spmd

    m = np.asarray(m)
    s_t = np.ascontiguousarray(np.asarray(s_t, dtype=np.float32))
    o = np.ascontiguousarray(np.asarray(o, dtype=np.float32))
    W1 = np.ascontiguousarray(np.asarray(W1, dtype=np.float32))
    b1 = np.ascontiguousarray(np.asarray(b1, dtype=np.float32))
    W2 = np.ascontiguousarray(np.asarray(W2, dtype=np.float32))
    b2 = np.ascontiguousarray(np.asarray(b2, dtype=np.float32))
    W3 = np.ascontiguousarray(np.asarray(W3, dtype=np.float32))
    b3 = np.ascontiguousarray(np.asarray(b3, dtype=np.float32))

    # one-hot masks, row q = j*8+n, particle = 16*p + j (fp16 0/1 exact),
    # packed [B, 96, 2, 128]: rows 0-95 at slot 0, rows 96-127 at slot 1
    # partitions 0-31 (matmul operands may only base at partition 0/32/64)
    mr = m.reshape(B, 128, JC).transpose(0, 2, 1)  # [B, j, p]
    ohf = (mr[:, :, None, :] == np.arange(N_M)[None, None, :, None])
    ohf = ohf.reshape(B, 128, 128).astype(np.float16)
    oh2 = np.zeros((B, 96, 2, 128), dtype=np.float16)
    oh2[:, :, 0, :] = ohf[:, 0:96, :]
    oh2[:, 0:32, 1, :] = ohf[:, 96:128, :]
    ohf = np.ascontiguousarray(oh2)

    nc = _get_bass()
    in_maps = []
    for c in range(NCORES):
        sl = slice(c * BL, (c + 1) * BL)
        in_maps.append({
            "s_t": s_t[sl], "ohjn": ohf[sl], "o": o[sl],
            "W1": W1, "b1": b1, "W2": W2, "b2": b2, "W3": W3, "b3": b3,
        })

    trace = bool(os.environ.get("BASS_KERNEL_TRACE"))
    if trace:
        _ensure_ntff_hook()
    res = run_bass_kernel_spmd(nc, in_maps, list(range(NCORES)), trace=trace)
    LAST_EXEC_NS = res.exec_time_ns
    LAST_RESULTS = res

    outp = np.concatenate([res.results[i]["out"] for i in range(NCORES)], axis=0)
    return outp.reshape(B, P, DIM).astype(np.float32, copy=False)


# revision 9
# speedup vs baseline: 1.1786x; 1.1786x over previous
"""Trainium2 Bass kernel for nn_Cond_PlanarTrans (conditional planar flow, MoE-routing).

Math (per batch b, particle i):
    w = relu(o @ W1.T + b1).reshape(B, 8, 64)
    u = relu(o @ W2.T + b2).reshape(B, 8, 64)
    bf = relu(o @ W3.T + b3).reshape(B, 8)
    n = m[b, i]
    pre = <s_t[b,i,:], w[b,n,:]> + bf[b,n]
    out[b,i,:] = s_t[b,i,:] + u[b,n,:] * tanh(pre)

Strategy: data-parallel over B across 8 cores (16 batches each). On each core:
  - tiny MLP computed once on the PE (weights transposed on-chip)
  - per-particle gathers are block-diagonal one-hot matmuls: 8 chunks (of 128
    particles each) share ONE matmul whose stationary operand is the 8 chunks'
    one-hot masks stacked [64, 128] (partition strips 0/64) and whose moving
    operand is a block-diagonal [64, 8*64] fp16 w table -> PSUM [128, 8, 64];
    the bf bias rides a separate tiny 8-col block-diag matmul
  - dot product on DVE: fp32 multiply straight off PSUM (fp16 product out),
    fp16 packed 2x reduce; bias add + tanh once per batch [128, 16]
  - a PE transpose turns tanh into rows; DVE scales the one-hot masks by tanh
    (exact: oh is 0/1); a second block-diag matmul yields u_m * tanh in PSUM
  - ACT evacuates the update to fp16, GPSIMD adds s_t, out stores ride the
    scalar-engine HWDGE ring while s_t loads use the sync ring.

Particle layout: partition p of a batch holds particles 16p..16p+15 (contiguous
4KB per partition -> full-rate DMA); chunk j of a batch = particles {16p+j}.
One-hot host layout: oh[b, j*8+n, p] = (m[b, 16p+j] == n), fp16 [B, 128, 128].
"""

import os
import sys

import numpy as np

B, P, DIM, N_M = 128, 2048, 64, 8
NCORES = 8
BL = B // NCORES  # batches per core
JC = 16           # chunks per batch (particle = 16*p + j)
G = 8             # chunks per group (one gather matmul per group)
NG = JC // G      # groups per batch

# tunables
NT = int(os.environ.get("PK_NT", "3"))    # s_t tile ring depth
OB = int(os.environ.get("PK_OB", "3"))    # out tile bufs
ADD_ENG = os.environ.get("PK_ADD", "gpsimd")  # gpsimd | vector

LAST_EXEC_NS = None
LAST_RESULTS = None

_CACHE = {}


def _import_concourse():
    try:
        import concourse.bass  # noqa: F401
    except ImportError:
        for p in ("/opt/trn_rl_repo", "/root/.axon_site/_ro/trn_rl_repo"):
            if os.path.isdir(p) and p not in sys.path:
                sys.path.insert(0, p)
        import concourse.bass  # noqa: F401


def _ensure_ntff_hook():
    """Provide antenv.axon_hooks (get/set_axon_ntff_profile_hook) if the image
    lacks it, wiring the NTFF profile capture directly to libaxon_pjrt.so."""
    try:
        from antenv.axon_hooks import get_axon_ntff_profile_hook  # noqa: F401
        return
    except ImportError:
        pass

    import contextlib
    import ctypes
    import types

    so_path = os.environ.get("AXON_PJRT_SO", "/opt/axon/libaxon_pjrt.so")
    hook = None
    if os.path.exists(so_path):
        lib = ctypes.CDLL(so_path)
        if hasattr(lib, "axon_start_nrt_profile"):
            lib.axon_start_nrt_profile.argtypes = [
                ctypes.POINTER(ctypes.c_int64),
                ctypes.c_size_t,
            ]
            lib.axon_start_nrt_profile.restype = ctypes.c_int64
            lib.axon_stop_nrt_profile.argtypes = [ctypes.c_char_p]
            lib.axon_stop_nrt_profile.restype = ctypes.c_int64

            @contextlib.contextmanager
            def hook(output_dir, device_ids):  # noqa: F811
                import jax

                jax.devices()
                if device_ids:
                    ids = (ctypes.c_int64 * len(device_ids))(*device_ids)
                    rc = lib.axon_start_nrt_profile(ids, len(device_ids))
                else:
                    rc = lib.axon_start_nrt_profile(None, 0)
                if rc != 0:
                    raise RuntimeError(f"axon_start_nrt_profile rc={rc}")
                try:
                    yield
                finally:
                    n = lib.axon_stop_nrt_profile(str(output_dir).encode())
                    print(f"profile: {n} file(s) written to {output_dir}")

    state = {"hook": hook}
    mod = types.ModuleType("antenv.axon_hooks")
    mod.get_axon_ntff_profile_hook = lambda: state["hook"]

    def _set(h):
        state["hook"] = h

    mod.set_axon_ntff_profile_hook = _set
    import antenv

    antenv.axon_hooks = mod
    sys.modules["antenv.axon_hooks"] = mod


def _build_bass():
    _import_concourse()
    from contextlib import ExitStack

    import concourse.bacc as bacc
    import concourse.bass as bass  # noqa: F401
    import concourse.tile as tile
    from concourse import mybir
    from concourse.masks import make_identity

    f32 = mybir.dt.float32
    f16 = mybir.dt.float16
    AF = mybir.ActivationFunctionType
    OP = mybir.AluOpType
    AX = mybir.AxisListType

    # Bacc (not plain Bass): its finalize() splits multi-sem waits into event
    # semaphores — TRN2 instructions carry at most one wait.
    nc = bacc.Bacc(None)

    s_t = nc.declare_dram_parameter("s_t", [BL, P, DIM], f32, isOutput=False)
    ohjn = nc.declare_dram_parameter("ohjn", [BL, 128, 128], f16, isOutput=False)
    o_in = nc.declare_dram_parameter("o", [BL, DIM], f32, isOutput=False)
    W1 = nc.declare_dram_parameter("W1", [N_M * DIM, DIM], f32, isOutput=False)
    b1 = nc.declare_dram_parameter("b1", [N_M * DIM], f32, isOutput=False)
    W2 = nc.declare_dram_parameter("W2", [N_M * DIM, DIM], f32, isOutput=False)
    b2 = nc.declare_dram_parameter("b2", [N_M * DIM], f32, isOutput=False)
    W3 = nc.declare_dram_parameter("W3", [N_M, DIM], f32, isOutput=False)
    b3 = nc.declare_dram_parameter("b3", [N_M], f32, isOutput=False)
    out = nc.declare_dram_parameter("out", [BL, P, DIM], f32, isOutput=True)

    with tile.TileContext(nc) as tc, ExitStack() as ctx:
        consts = ctx.enter_context(tc.tile_pool(name="consts", bufs=1))

        # ---------- phase 0: constants + per-batch MLP ----------
        ident = consts.tile([128, 128], f32)
        make_identity(nc, ident)
        ident16 = consts.tile([128, 128], f16)
        make_identity(nc, ident16)
        ones_row = consts.tile([1, 128], f32)
        nc.vector.memset(ones_row, 1.0)

        w1_sb = consts.tile([128, 4, DIM], f32)
        nc.sync.dma_start(out=w1_sb, in_=W1[:].rearrange("(q r) k -> r q k", r=128))
        w2_sb = consts.tile([128, 4, DIM], f32)
        nc.sync.dma_start(out=w2_sb, in_=W2[:].rearrange("(q r) k -> r q k", r=128))
        w3_sb = consts.tile([N_M, DIM], f32)
        nc.sync.dma_start(out=w3_sb, in_=W3[:])
        b1_sb = consts.tile([1, N_M * DIM], f32)
        nc.sync.dma_start(out=b1_sb, in_=b1[:].rearrange("(a n) -> a n", a=1))
        b2_sb = consts.tile([1, N_M * DIM], f32)
        nc.sync.dma_start(out=b2_sb, in_=b2[:].rearrange("(a n) -> a n", a=1))
        b3_sb = consts.tile([1, N_M], f32)
        nc.sync.dma_start(out=b3_sb, in_=b3[:].rearrange("(a n) -> a n", a=1))
        o_sb = consts.tile([BL, DIM], f32)
        nc.sync.dma_start(out=o_sb, in_=o_in[:])

        with tc.tile_pool(name="mlp_ps", bufs=2, space="PSUM") as mlp_ps:
            # transposes: oT [64, BL]; W1T/W2T [64, 512]; W3T [64, 8]
            oT = consts.tile([DIM, BL], f32)
            pt_o = mlp_ps.tile([DIM, BL], f32, tag="pt")
            nc.tensor.transpose(pt_o, o_sb, ident[0:BL, 0:BL])
            nc.vector.tensor_copy(oT, pt_o)

            w1T = consts.tile([DIM, N_M * DIM], f32)
            w2T = consts.tile([DIM, N_M * DIM], f32)
            for src, dst in ((w1_sb, w1T), (w2_sb, w2T)):
                for q in range(4):
                    pt = mlp_ps.tile([DIM, 128], f32, tag="pt")
                    nc.tensor.transpose(pt, src[:, q, :], ident)
                    nc.vector.tensor_copy(dst[:, q * 128:(q + 1) * 128], pt)
            w3T = consts.tile([DIM, N_M], f32)
            pt_3 = mlp_ps.tile([DIM, N_M], f32, tag="pt")
            nc.tensor.transpose(pt_3, w3_sb, ident[0:N_M, 0:N_M])
            nc.vector.tensor_copy(w3T, pt_3)

            # MLP: x_all = relu(o @ W.T + b), bias preloaded via ones-matmul
            w_all = consts.tile([BL, N_M * DIM], f32)
            u_all = consts.tile([BL, N_M * DIM], f32)
            bf_all = consts.tile([BL, N_M], f32)
            for bsb, wT, dst in (
                (b1_sb, w1T, w_all),
                (b2_sb, w2T, u_all),
                (b3_sb, w3T, bf_all),
            ):
                n_cols = dst.shape[-1]
                ps = mlp_ps.tile([BL, n_cols], f32, tag="mlp")
                nc.tensor.matmul(ps, lhsT=ones_row[0:1, 0:BL], rhs=bsb,
                                 start=True, stop=False)
                nc.tensor.matmul(ps, lhsT=oT, rhs=wT, start=False, stop=True)
                nc.scalar.activation(out=dst, in_=ps, func=AF.Relu)

        # Block-diagonal fp16 gather tables, built via a DRAM bounce
        # (partition-reshape) with inline fp32->fp16 SWDGE cast, then
        # replicated to partition strip 64 (matmul operands base at 0/64):
        #   wb8_sb[8jj+n, b, 64jj+c] = w[b, n, c]
        #   u8_sb [8jj+n, b, 64jj+c] = u[b, n, c]
        #   bft_sb[8jj+n, b, jj]     = bf[b, n]
        w_dram = nc.dram_tensor("w_scratch", [BL, N_M * DIM], f32)
        u_dram = nc.dram_tensor("u_scratch", [BL, N_M * DIM], f32)
        bf_dram = nc.dram_tensor("bf_scratch", [BL, N_M], f32)
        nc.sync.dma_start(out=w_dram[:], in_=w_all)
        nc.sync.dma_start(out=u_dram[:], in_=u_all)
        nc.sync.dma_start(out=bf_dram[:], in_=bf_all)

        wb8_sb = consts.tile([128, BL, G * DIM], f16)
        u8_sb = consts.tile([128, BL, G * DIM], f16)
        bft_sb = consts.tile([128, BL, G], f16)
        nc.gpsimd.memset(wb8_sb[0:64], 0.0)
        nc.gpsimd.memset(u8_sb[0:64], 0.0)
        nc.gpsimd.memset(bft_sb[0:64], 0.0)
        for jj in range(G):
            nc.gpsimd.dma_start(
                out=wb8_sb[8 * jj:8 * jj + 8, :, DIM * jj:DIM * jj + DIM],
                in_=w_dram[:].rearrange("b (n k) -> n b k", n=N_M),
            )
            nc.gpsimd.dma_start(
                out=u8_sb[8 * jj:8 * jj + 8, :, DIM * jj:DIM * jj + DIM],
                in_=u_dram[:].rearrange("b (n k) -> n b k", n=N_M),
            )
            nc.gpsimd.dma_start(
                out=bft_sb[8 * jj:8 * jj + 8, :, jj:jj + 1],
                in_=bf_dram[:].rearrange("b (n a) -> n b a", a=1),
            )
        nc.sync.dma_start(out=wb8_sb[64:128], in_=wb8_sb[0:64])
        nc.sync.dma_start(out=u8_sb[64:128], in_=u8_sb[0:64])
        nc.sync.dma_start(out=bft_sb[64:128], in_=bft_sb[0:64])

        # ---------- phase 1: main loop ----------
        stpool = ctx.enter_context(tc.tile_pool(name="stpool", bufs=NT))
        ohpool = ctx.enter_context(tc.tile_pool(name="ohpool", bufs=3))
        outpool = ctx.enter_context(tc.tile_pool(name="outpool", bufs=OB))
        prpool = ctx.enter_context(tc.tile_pool(name="prpool", bufs=3))
        updpool = ctx.enter_context(tc.tile_pool(name="updpool", bufs=4))
        smpool = ctx.enter_context(tc.tile_pool(name="smpool", bufs=3))
        pswpool = ctx.enter_context(tc.tile_pool(name="psw", bufs=2, space="PSUM"))
        psupool = ctx.enter_context(tc.tile_pool(name="psu", bufs=2, space="PSUM"))
        psbpool = ctx.enter_context(tc.tile_pool(name="psb", bufs=2, space="PSUM"))
        psthpool = ctx.enter_context(tc.tile_pool(name="psth", bufs=2, space="PSUM"))

        add_eng = nc.gpsimd if ADD_ENG == "gpsimd" else nc.vector

        def emit_phase2_group(prev, g):
            pb, pst, pohs, pout = prev
            psu = psupool.tile([128, G, DIM], f32, tag="psu")
            nc.tensor.matmul(
                psu, lhsT=pohs[64 * g:64 * g + 64, :],
                rhs=u8_sb[64 * g:64 * g + 64, pb, :],
                start=True, stop=True,
            )
            upd = updpool.tile([128, G, DIM], f32, tag="upd")
            nc.scalar.activation(out=upd, in_=psu, func=AF.Copy)
            add_eng.tensor_tensor(
                out=pout[:, G * g:G * g + G, :],
                in0=upd, in1=pst[:, G * g:G * g + G, :], op=OP.add,
            )
            if g == NG - 1:
                nc.sync.dma_start(
                    out=out[pb].rearrange("(p j) k -> p j k", j=JC),
                    in_=pout,
                )

        prev = None  # (b, s_t, ohs, out_tile)
        for b in range(BL):
            st = stpool.tile([128, JC, DIM], f32, tag="st")
            nc.sync.dma_start(
                out=st, in_=s_t[b].rearrange("(p j) k -> p j k", j=JC),
            )
            oht = ohpool.tile([128, 128], f16, tag="oh")
            nc.sync.dma_start(out=oht, in_=ohjn[b])
            outt = outpool.tile([128, JC, DIM], f32, tag="outt")
            pre_b = smpool.tile([128, JC], f32, tag="pre")

            psbf = psbpool.tile([128, JC], f32, tag="psbf")
            for g in range(NG):
                nc.tensor.matmul(
                    psbf[:, G * g:G * g + G],
                    lhsT=oht[64 * g:64 * g + 64, :],
                    rhs=bft_sb[64 * g:64 * g + 64, b, :],
                    start=True, stop=True,
                )

            for g in range(NG):
                psw = pswpool.tile([128, G, DIM], f32, tag="psw")
                nc.tensor.matmul(
                    psw, lhsT=oht[64 * g:64 * g + 64, :],
                    rhs=wb8_sb[64 * g:64 * g + 64, b, :],
                    start=True, stop=True,
                )
                pr = prpool.tile([128, G, DIM], f16, tag="pr")
                nc.vector.tensor_tensor(
                    out=pr, in0=st[:, G * g:G * g + G, :], in1=psw, op=OP.mult,
                )
                nc.vector.reduce_sum(
                    out=pre_b[:, G * g:G * g + G], in_=pr, axis=AX.X,
                )
                if prev is not None:
                    emit_phase2_group(prev, g)

            pre2 = smpool.tile([128, JC], f32, tag="pre2")
            nc.vector.tensor_tensor(out=pre2, in0=pre_b, in1=psbf, op=OP.add)
            th_b = smpool.tile([128, JC], f32, tag="th")
            nc.scalar.activation(out=th_b, in_=pre2, func=AF.Tanh)
            # replicate along n then transpose: psth[(j n), p] = tanh(pre[p, j])
            thx = smpool.tile([128, JC, N_M], f16, tag="thx")
            th_src = bass.AP(
                tensor=th_b.tensor,
                offset=th_b.offset,
                ap=[th_b.ap[0], [th_b.ap[1][0], JC], [0, N_M]],
            )
            nc.vector.tensor_copy(thx, th_src)
            psth = psthpool.tile([128, 128], f16, tag="psth")
            nc.tensor.transpose(psth, thx.rearrange("p j n -> p (j n)"), ident16)
            ohs = smpool.tile([128, 128], f16, tag="ohs")
            nc.vector.tensor_tensor(out=ohs, in0=oht, in1=psth, op=OP.mult)

            prev = (b, st, ohs, outt)

        for g in range(NG):
            emit_phase2_group(prev, g)

    nc.finalize()
    return nc


def _get_bass():
    if "nc" not in _CACHE:
        _CACHE["nc"] = _build_bass()
    return _CACHE["nc"]


def kernel(m, s_t, o, W1, b1, W2, b2, W3, b3):
    global LAST_EXEC_NS, LAST_RESULTS
    _import_concourse()
    from concourse.bass_utils import run_bass_kernel_spmd

    m = np.asarray(m)
    s_t = np.ascontiguousarray(np.asarray(s_t, dtype=np.float32))
    o = np.ascontiguousarray(np.asarray(o, dtype=np.float32))
    W1 = np.ascontiguousarray(np.asarray(W1, dtype=np.float32))
    b1 = np.ascontiguousarray(np.asarray(b1, dtype=np.float32))
    W2 = np.ascontiguousarray(np.asarray(W2, dtype=np.float32))
    b2 = np.ascontiguousarray(np.asarray(b2, dtype=np.float32))
    W3 = np.ascontiguousarray(np.asarray(W3, dtype=np.float32))
    b3 = np.ascontiguousarray(np.asarray(b3, dtype=np.float32))

    # one-hot masks, row q = j*8+n, particle = 16*p + j (fp16 0/1 exact)
    mr = m.reshape(B, 128, JC).transpose(0, 2, 1)  # [B, j, p]
    ohf = (mr[:, :, None, :] == np.arange(N_M)[None, None, :, None])
    ohf = np.ascontiguousarray(ohf.reshape(B, 128, 128).astype(np.float16))

    nc = _get_bass()
    in_maps = []
    for c in range(NCORES):
        sl = slice(c * BL, (c + 1) * BL)
        in_maps.append({
            "s_t": s_t[sl], "ohjn": ohf[sl], "o": o[sl],
            "W1": W1, "b1": b1, "W2": W2, "b2": b2, "W3": W3, "b3": b3,
        })

    trace = bool(os.environ.get("BASS_KERNEL_TRACE"))
    if trace:
        _ensure_ntff_hook()
    res = run_bass_kernel_spmd(nc, in_maps, list(range(NCORES)), trace=trace)
    LAST_EXEC_NS = res.exec_time_ns
    LAST_RESULTS = res

    outp = np.concatenate([res.results[i]["out"] for i in range(NCORES)], axis=0)
    return outp.reshape(B, P, DIM).astype(np.float32, copy=False)
